# revision 24
# baseline (speedup 1.0000x reference)
"""GCNConv custom kernel for Trainium2 (8 NeuronCores, SPMD row-sharded).

Math (matches the reference exactly):
    A = max(scatter(edges), scatter(edges).T) + I        # dense [N, N]
    deg = A.sum(axis=1); d = 1/sqrt(deg + EPS)
    out = (d[:,None] * A * d[None,:]) @ x @ W + b

Strategy (memory-regime): the dedup'd symmetric edge set IS the dense
adjacency's structure, so the host packs each device's column strip
A[:, dev*1024:(dev+1)*1024] as a dense fp8 bitmap (entries 0/1/2, exact in
fp8), column-half-major with tapered chunk sizes, and the device streams
it across THREE concurrent DMA queues (SP + Activation HWDGE, Pool SWDGE)
for ~3x effective HBM bandwidth.  The column-scaled features z = d (.) x
ship as an fp8 hi+lo pair (z = zhi+zlo to ~2^-8 relative precision —
input quantization, same byte count as f16 x), and the PE chases the A
stream with fp8 DoubleRow matmuls (paired j-tiles, 0.5 cyc/col)
accumulating aggT[c, li] in PSUM; the PE DoubleRow roofline (~13.7us) is
the binding constraint.  Each column half gets aggT @ W with the bias
folded into the same PSUM group via a rank-1 (1/d_my (x) b) matmul so the
d_my row scale (an Activation per-partition scale pointer on the
PSUM->SBUF copy) restores it exactly; half 0's W-apply hides inside half
1's stream.  No collectives: every device keeps the full degree vector
(host bincount of the same edge set it already dedups).
"""

import sys

for _p in ("/root/.axon_site", "/root/.axon_site/_ro/trn_rl_repo", "/opt/trn_rl_repo"):
    if _p not in sys.path:
        sys.path.append(_p)

import bisect

import numpy as np

import concourse.bass as bass
import concourse.mybir as mybir
import concourse.tile as tile
from concourse import bacc
from concourse import bass_utils

F32 = mybir.dt.float32
F16 = mybir.dt.float16
F8 = mybir.dt.float8e4

N = 8192
D = 128
NDEV = 8
NSH = N // NDEV          # rows (li) per device
NT = N // 128            # j tiles
NL = NSH // 128          # li tiles
EPS = 1e-5
NWARM = 6                # PE p-state warmup matmuls (512-wide)

# A-stream chunks per column half: tapered so the first aggregation
# quantum starts early and the last one finishes with a short chain
CH_H0 = [(0, 2), (2, 2)] + [(4 + 4 * i, 4) for i in range(15)]
CH_H1 = [(4 * i, 4) for i in range(15)] + [(60, 2), (62, 2)]
CHUNKS = {0: CH_H0, 1: CH_H1}
# z tile groups (j-tiles per DMA), first small for fast pipeline start
ZG = [(0, 4), (4, 12), (16, 16), (32, 16), (48, 16)]
ZG_BASE = [g[0] for g in ZG]


def _build_program():
    nc = bacc.Bacc("TRN2", target_bir_lowering=False, debug=False,
                   num_devices=NDEV)

    a8_d = nc.dram_tensor("a8", [128, 2 * NT * 512], F8, kind="ExternalInput")
    zhi_d = nc.dram_tensor("zhi", [128, NT * D], F8, kind="ExternalInput")
    zlo_d = nc.dram_tensor("zlo", [128, NT * D], F8, kind="ExternalInput")
    dmy_d = nc.dram_tensor("dmy", [128, NL], F32, kind="ExternalInput")
    rd8_d = nc.dram_tensor("rd8", [NL, NSH], F16, kind="ExternalInput")
    w_d = nc.dram_tensor("w16", [128, D], F16, kind="ExternalInput")
    b_d = nc.dram_tensor("b8", [NL, D], F16, kind="ExternalInput")
    out_d = nc.dram_tensor("out", [128, NL * D], F16, kind="ExternalOutput")

    with tile.TileContext(nc) as tc:
        with tc.tile_pool(name="c", bufs=1) as cpool:
            ach = {}
            for h in range(2):
                for (t0, nt_) in CHUNKS[h]:
                    ach[(h, t0)] = cpool.tile([128, nt_, 512], F8,
                                              tag=f"a{h}_{t0}",
                                              name=f"a{h}_{t0}")
            zhi = [cpool.tile([128, g[1], D], F8, tag=f"zh{i}",
                              name=f"zh{i}") for i, g in enumerate(ZG)]
            zlo = [cpool.tile([128, g[1], D], F8, tag=f"zl{i}",
                              name=f"zl{i}") for i, g in enumerate(ZG)]
            dmy = cpool.tile([128, NL], F32)
            rd8 = cpool.tile([NL, NSH], F16)
            w16 = cpool.tile([128, D], F16)
            b8 = cpool.tile([NL, D], F16)

            def dma_a(eng, h, t0):
                nt_ = dict(CHUNKS[h])[t0]
                base = (h * NT + t0) * 512
                eng.dma_start(out=ach[(h, t0)][:],
                              in_=a8_d.ap()[:, base:base + nt_ * 512])

            def dma_z(eng, zt, zd, g):
                t0, nt_ = ZG[g]
                eng.dma_start(out=zt[g][:],
                              in_=zd.ap()[:, t0 * D:(t0 + nt_) * D])

            # ---- three concurrent DMA queues.  A greedy scheduler assigns
            # each transfer to the queue that can deliver it earliest given
            # its need time (PE consumes ~107ns/j-tile per half from ~T0).
            T0 = 3.3e3
            RATE = 107.0
            HALF = RATE * NT
            items = []   # (need_ns, bytes, emit_fn)
            for h in range(2):
                for (t0, nt_) in CHUNKS[h]:
                    items.append((T0 + h * HALF + RATE * t0, nt_ * 512 * 128,
                                  lambda e, h=h, t0=t0: dma_a(e, h, t0)))
            for g, (t0, nt_) in enumerate(ZG):
                zb = nt_ * D * 128
                items.append((T0 + RATE * t0 - 200, zb,
                              lambda e, g=g: dma_z(e, zhi, zhi_d, g)))
                items.append((T0 + RATE * t0 - 100, zb,
                              lambda e, g=g: dma_z(e, zlo, zlo_d, g)))
            tw = T0 + HALF
            items.append((tw, D * D * 2,
                          lambda e: e.dma_start(out=w16[:], in_=w_d.ap())))
            items.append((tw, NL * 128 * 4,
                          lambda e: e.dma_start(out=dmy[:], in_=dmy_d.ap())))
            items.append((tw, NL * NSH * 2,
                          lambda e: e.dma_start(out=rd8[:], in_=rd8_d.ap())))
            items.append((tw, NL * D * 2,
                          lambda e: e.dma_start(out=b8[:], in_=b_d.ap())))
            items.sort(key=lambda it: it[0])

            DGE_SEM = 650.0 + 900.0
            queues = {  # engine: [clock_ns, per-item fixed overhead]
                "sp": [200.0, 123.0],
                "act": [1800.0, 123.0],     # behind the one-time table load
                "pool": [100.0, 1040.0],    # SWDGE holds the Pool engine
            }
            engs = {"sp": nc.sync, "act": nc.scalar, "pool": nc.gpsimd}
            plan = {q: [] for q in queues}
            for need, nbytes, emit in items:
                tr = nbytes / 360.0  # ns at full aggregate DMA bus rate
                # deadline heuristic: among queues that can deliver by the
                # need time, take the most-loaded (save fast queues for
                # tight deadlines); else take the earliest delivery
                fits, best = [], None
                for q, (clk, ovh) in queues.items():
                    t = clk + ovh + tr
                    if t + 1550.0 <= need:
                        fits.append((clk, t, q))
                    if best is None or t < best[0]:
                        best = (t, q)
                if fits:
                    _, t, q = min(fits)
                else:
                    t, q = best
                queues[q][0] = t
                plan[q].append(emit)
            for q in ("sp", "act", "pool"):
                for emit in plan[q]:
                    emit(engs[q])

            with (
                tc.tile_pool(name="psum_w", bufs=1, space="PSUM") as pwarm,
                tc.tile_pool(name="psum_a", bufs=2, space="PSUM") as pagg,
                tc.tile_pool(name="psum_o", bufs=3, space="PSUM") as pout,
            ):
                # ---- PE p-state warmup (content is garbage zeros)
                warm = cpool.tile([128, 512], F16)
                nc.vector.memset(warm[:], 0.0)
                wpsum = pwarm.tile([128, 512], F32)
                for i in range(NWARM):
                    nc.tensor.matmul(out=wpsum[:], lhsT=warm[:, :D],
                                     rhs=warm[:], start=True, stop=True)

                def zslice(t0):
                    g = bisect.bisect_right(ZG_BASE, t0) - 1
                    return g, t0 - ZG_BASE[g]

                pas = [pagg.tile([128, 512], F32, tag=f"pa{h}", name=f"pa{h}")
                       for h in range(2)]

                def agg_chunk(h, ci):
                    t0, nt_ = CHUNKS[h][ci]
                    for pi in range(nt_ // 2):
                        g, jj = zslice(t0 + 2 * pi)
                        for zs in (zhi, zlo):
                            nc.tensor.matmul(
                                out=pas[h][:],
                                lhsT=zs[g][:, jj:jj + 2, :],
                                rhs=ach[(h, t0)][:, 2 * pi:2 * pi + 2, :],
                                perf_mode=mybir.MatmulPerfMode.DoubleRow,
                                start=(ci == 0 and pi == 0 and zs is zhi),
                                stop=(ci == len(CHUNKS[h]) - 1
                                      and pi == nt_ // 2 - 1 and zs is zlo))

                def w_apply(h):
                    # aggT pieces (copied on the idle DVE) so W starts
                    # early; one o16 tile + out DMA per piece so each leaves
                    # as soon as it is written; h1's last pieces taper so
                    # the final chain is short
                    pieces = [(0, 2), (2, 2)]
                    for q, (p0, np_) in enumerate(pieces):
                        aggT = cpool.tile([128, np_ * D], F16,
                                          tag=f"aggT{h}{q}",
                                          name=f"aggT{h}{q}")
                        nc.vector.tensor_copy(
                            out=aggT[:],
                            in_=pas[h][:, p0 * D:(p0 + np_) * D])
                        o16 = cpool.tile([128, np_, D], F16, tag=f"o{h}{q}",
                                         name=f"o{h}{q}")
                        for i in range(np_):
                            lt = h * 4 + p0 + i
                            po = pout.tile([128, D], F32, tag="po", name="po")
                            nc.tensor.matmul(
                                out=po[:], lhsT=aggT[:, i * D:(i + 1) * D],
                                rhs=w16[:], start=True, stop=False)
                            nc.tensor.matmul(
                                out=po[:], lhsT=rd8[:, lt * D:(lt + 1) * D],
                                rhs=b8[:], start=False, stop=True)
                            nc.scalar.activation(
                                out=o16[:, i, :], in_=po[:],
                                func=mybir.ActivationFunctionType.Copy,
                                scale=dmy[:, lt:lt + 1])
                        base = (h * 4 + p0) * D
                        nc.sync.dma_start(
                            out=out_d.ap()[:, base:base + np_ * D],
                            in_=o16[:])

                for ci in range(len(CH_H0)):
                    agg_chunk(0, ci)
                # h1's first chunks before W-h0 so the PE never stalls on
                # the aggT0 copies
                agg_chunk(1, 0)
                agg_chunk(1, 1)
                w_apply(0)
                for ci in range(2, len(CH_H1)):
                    agg_chunk(1, ci)
                w_apply(1)

    nc.compile()
    return nc


def _host_prep(x, edge_index, weight, bias):
    """Pack inputs: dense fp8 adjacency column strips (pure layout change of
    the dedup'd edge set), the degree-scaled features as an fp8 hi+lo pair
    (input quantization, z = zhi + zlo to ~2^-8), and the d-derived scale
    vectors, all in the partition-major layouts the device DMAs directly
    into SBUF."""
    f8 = mybir.dt.np(F8)
    a = np.asarray(edge_index[0], dtype=np.int64)
    b = np.asarray(edge_index[1], dtype=np.int64)

    adj = np.zeros((N, N), dtype=np.uint8)
    adj[a, b] = 1
    adj |= adj.T                                   # symmetrize (max of 0/1)
    idx = np.arange(N)
    adj[idx, idx] += 1                             # self loops (may yield 2)

    deg = adj.sum(axis=1, dtype=np.int64)
    d = (1.0 / np.sqrt(deg.astype(np.float64) + EPS)).astype(np.float32)

    a8 = adj.astype(f8)                            # 0/1/2 exact in fp8

    z32 = np.asarray(x, dtype=np.float32) * d[:, None]
    zh = z32.astype(f8)
    zl = (z32 - zh.astype(np.float32)).astype(f8)

    def pack_pm(arr):                              # [N, D] -> [128, NT*D]
        return np.ascontiguousarray(
            arr.reshape(NT, 128, D).transpose(1, 0, 2)).reshape(128, NT * D)

    w16 = np.ascontiguousarray(np.asarray(weight, dtype=np.float16))
    b8 = np.ascontiguousarray(
        np.broadcast_to(np.asarray(bias, dtype=np.float16), (NL, D)))

    in_maps = []
    for dev in range(NDEV):
        strip = a8[:, dev * NSH:(dev + 1) * NSH]
        # [j, li] -> [p=j%128, h=li//512, t=j//128, c=li%512], C-contiguous
        a8p = np.ascontiguousarray(
            strip.reshape(NT, 128, 2, 512).transpose(1, 2, 0, 3)).reshape(
                128, 2 * NT * 512)
        dloc = d[dev * NSH:(dev + 1) * NSH]
        dmyp = np.ascontiguousarray(dloc.reshape(NL, 128).T)
        rd8p = np.zeros((NL, NSH), dtype=np.float16)
        for q in range(NL):
            rd8p[q, q * 128:(q + 1) * 128] = \
                (1.0 / dloc[q * 128:(q + 1) * 128]).astype(np.float16)
        in_maps.append({
            "a8": a8p, "zhi": pack_pm(zh), "zlo": pack_pm(zl),
            "dmy": dmyp, "rd8": rd8p, "w16": w16, "b8": b8,
        })
    return in_maps


_prog_cache = {}


def _get_program():
    key = (N, D, NDEV)
    if key not in _prog_cache:
        _prog_cache[key] = _build_program()
    return _prog_cache[key]


last_results = None
TRACE = False


def kernel(x, edge_index, weight, bias):
    global last_results
    in_maps = _host_prep(x, edge_index, weight, bias)
    nc = _get_program()
    res = bass_utils.run_bass_kernel_spmd(
        nc, in_maps, core_ids=list(range(NDEV)), trace=TRACE)
    last_results = res
    parts = []
    for i in range(NDEV):
        o = np.asarray(res.results[i]["out"], dtype=np.float32)
        parts.append(o.reshape(128, NL, D).transpose(1, 0, 2).reshape(NSH, D))
    return np.concatenate(parts, axis=0)
